# revision 14
# baseline (speedup 1.0000x reference)
"""Trainium2 Bass kernel for the batched ConstituencyTreeLSTM cell.

Data-parallel across 8 NeuronCores: each core processes 12500 nodes
(padded to 12544 = 98 micro-tiles of 128 nodes, grouped 7 per "group").
Per 128-node micro-tile:

  leaf:  stationary chunks [e;h_prev]^T (4) + [tag;tagp;1]^T (1) vs fused
         weights [We;Uh] / [Wt;Wtp;b]  ->  bf16 PSUM [128, i|o|fl|u]
         c1 = sig(i)*tanh(u) + sig(fl)*c_prev ; h1 = sig(o)*tanh(c1)
  node:  h1^T obtained on-chip (rows 0:128 via DMA xbar transpose, rows
         128:168 via PE transpose); stationary chunks [tag;tagp;1]^T,
         h1^T, [h1^T;k^T], k^T (4) vs [Wt_n;Wtp_n;b_n] / Uh_n / Uk_n
         (u2 gate zero-padded in Uk)  ->  bf16 PSUM [128, i2|o2|fl2|fd2|u2]
         c2 = sig(i2)*tanh(u2) + sig(fd2)*q + sig(fl2)*c1
         h2 = sig(o2)*tanh(c2);  out = [h2|c2]

All matmul inputs are bf16 (bf16 PSUM accumulation); biases are folded in
via a ones-row. Elementwise runs bf16 on DVE, activations on ACT.
Pipeline: node block of group g runs two iterations after its leaf block
so the h1 -> transpose chain never stalls the TensorEngine (keeps HAM warm).
"""

import os
import sys

import numpy as np

try:
    import concourse.bass as bass  # noqa: F401
except Exception:  # pragma: no cover - fallback for bare environments
    for p in (
        "/root/.axon_site",
        "/root/.axon_site/_ro/trn_rl_repo",
        "/root/.axon_site/_ro/pypackages",
        "/opt/trn_rl_repo",
        "/opt/pypackages",
    ):
        if os.path.isdir(p) and p not in sys.path:
            sys.path.append(p)
    import concourse.bass as bass  # noqa: F401

import ml_dtypes
import concourse.mybir as mybir
import concourse.tile as tile
from concourse import bacc
from concourse.bass_utils import run_bass_kernel_spmd
from concourse.masks import make_identity

BF16 = ml_dtypes.bfloat16

N_CORES = 8
N = 100000
NPER = N // N_CORES            # 12500
MICRO = 128                    # nodes per matmul tile (stationary free dim)
GRP = 7                        # micro tiles per group
GNODES = MICRO * GRP           # 896
NGRP = 14                      # groups per core
NPAD = NGRP * GNODES           # 12544
M = 168                        # mem dim

ROWS_EH = [(0, 128), (128, 256), (256, 384), (384, 468)]  # [e;h_prev] chunks

F32 = mybir.dt.float32
BF = mybir.dt.bfloat16
SIGF = mybir.ActivationFunctionType.Sigmoid
TANHF = mybir.ActivationFunctionType.Tanh

_compiled = None
LAST_RESULT = None


def _build(ngrp=NGRP):
    npad = ngrp * GNODES
    nc = bacc.Bacc("TRN2", target_bir_lowering=False, debug=False,
                   num_devices=N_CORES)

    xbig_d = nc.dram_tensor("xbig", [640, npad], BF, kind="ExternalInput")
    tt1T_d = nc.dram_tensor("tt1T", [101, npad], BF, kind="ExternalInput")
    ktail_d = nc.dram_tensor("ktail", [40, npad], BF, kind="ExternalInput")
    cq_d = nc.dram_tensor("cq", [npad, 336], BF, kind="ExternalInput")
    w_eh_d = nc.dram_tensor("w_eh", [468, 672], BF, kind="ExternalInput")
    w_ttl_d = nc.dram_tensor("w_ttl", [101, 672], BF, kind="ExternalInput")
    w_ttn_d = nc.dram_tensor("w_ttn", [101, 840], BF, kind="ExternalInput")
    w_b_d = nc.dram_tensor("w_b", [128, 840], BF, kind="ExternalInput")
    w_c_d = nc.dram_tensor("w_c", [128, 840], BF, kind="ExternalInput")
    w_d_d = nc.dram_tensor("w_d", [80, 840], BF, kind="ExternalInput")
    out_d = nc.dram_tensor("out", [npad, 336], BF, kind="ExternalOutput")

    with tile.TileContext(nc) as tc:
        from contextlib import ExitStack
        with ExitStack() as ctx:
            wpool = ctx.enter_context(tc.tile_pool(name="w", bufs=1))
            spool = ctx.enter_context(tc.tile_pool(name="s", bufs=5))
            opool = ctx.enter_context(tc.tile_pool(name="o", bufs=2))
            gpool = ctx.enter_context(tc.tile_pool(name="g", bufs=2))
            c1pool = ctx.enter_context(tc.tile_pool(name="c1p", bufs=3))
            plpool = ctx.enter_context(
                tc.tile_pool(name="pl", bufs=2, space="PSUM"))
            pnpool = ctx.enter_context(
                tc.tile_pool(name="pn", bufs=2, space="PSUM"))

            # ---- constants / weights (resident) ----
            ident = wpool.tile([128, 128], BF, tag="ident")
            make_identity(nc, ident[:])
            wleaf = []
            for i, (r0, r1) in enumerate(ROWS_EH):
                t = wpool.tile([r1 - r0, 672], BF, tag=f"weh{i}")
                nc.gpsimd.dma_start(t[:], w_eh_d[r0:r1, :])
                wleaf.append(t)
            wttl = wpool.tile([101, 672], BF, tag="wttl")
            nc.gpsimd.dma_start(wttl[:], w_ttl_d[:, :])
            wttn = wpool.tile([101, 840], BF, tag="wttn")
            nc.gpsimd.dma_start(wttn[:], w_ttn_d[:, :])
            wb = wpool.tile([128, 840], BF, tag="wb")
            nc.gpsimd.dma_start(wb[:], w_b_d[:, :])
            wc = wpool.tile([128, 840], BF, tag="wc")
            nc.gpsimd.dma_start(wc[:], w_c_d[:, :])
            wd = wpool.tile([80, 840], BF, tag="wd")
            nc.gpsimd.dma_start(wd[:], w_d_d[:, :])

            sstate = {}
            gstate = {}

            def load_group(g):
                cs = g * GNODES
                EK = spool.tile([128, 5, GNODES], BF, tag="EK")
                nc.gpsimd.dma_start(
                    EK[:],
                    xbig_d[:, cs:cs + GNODES].rearrange("(c p) n -> p c n",
                                                        p=128))
                TTs = spool.tile([101, GNODES], BF, tag="TT")
                nc.gpsimd.dma_start(TTs[:], tt1T_d[:, cs:cs + GNODES])
                Dt = spool.tile([80, GNODES], BF, tag="D")
                nc.gpsimd.dma_start(Dt[40:80, :], ktail_d[:, cs:cs + GNODES])
                Bt = spool.tile([128, GNODES], BF, tag="B")
                CQt = spool.tile([128, GRP, 336], BF, tag="CQ")
                nc.gpsimd.dma_start(
                    CQt[:],
                    cq_d[cs:cs + GNODES, :].rearrange("(m p) f -> p m f",
                                                      p=128))
                sstate[g] = dict(EK=EK, TT=TTs, D=Dt, B=Bt, CQt=CQt)

            def leaf_block(g):
                st = sstate[g]
                EK, TTs = st["EK"], st["TT"]
                sgl = gpool.tile([128, GRP, 504], BF, tag="sgl")
                tul = gpool.tile([128, GRP, 168], BF, tag="tul")
                for j in range(GRP):
                    c0 = j * MICRO
                    chunks = [(EK[0:r1 - r0, i, c0:c0 + MICRO], wleaf[i])
                              for i, (r0, r1) in enumerate(ROWS_EH)]
                    chunks.append((TTs[:, c0:c0 + MICRO], wttl))
                    P = plpool.tile([128, 1024], F32, tag="psl")
                    for ci, (X, W) in enumerate(chunks):
                        first, last = ci == 0, ci == len(chunks) - 1
                        nc.tensor.matmul(P[:, 0:504], X,
                                         W[:, 0:504], start=first, stop=last)
                        nc.tensor.matmul(P[:, 512:680], X,
                                         W[:, 504:672], start=first, stop=last)
                    nc.scalar.activation(sgl[:, j, :], P[:, 0:504], SIGF)
                    nc.scalar.activation(tul[:, j, :], P[:, 512:680], TANHF)
                gstate[g] = dict(sgl=sgl, tul=tul)

            def chain_block(g):
                st = sstate[g]
                gs = gstate[g]
                sgl, tul = gs["sgl"], gs["tul"]
                CQt, Bt = st["CQt"], st["B"]
                t1 = gpool.tile([128, GRP, 168], BF, tag="tmpA")
                nc.vector.tensor_mul(t1[:], sgl[:, :, 0:168], tul[:])
                t2 = gpool.tile([128, GRP, 168], BF, tag="tmpB")
                nc.vector.tensor_mul(t2[:], sgl[:, :, 336:504],
                                     CQt[:, :, 0:168])
                c1t = c1pool.tile([128, GRP, 168], BF, tag="c1")
                nc.vector.tensor_add(c1t[:], t1[:], t2[:])
                tc1 = gpool.tile([128, GRP, 168], BF, tag="tc1")
                nc.scalar.activation(tc1[:], c1t[:], TANHF)
                h1t = c1pool.tile([128, GRP, 168], BF, tag="h1")
                nc.vector.tensor_mul(h1t[:], sgl[:, :, 168:336], tc1[:])
                for j in range(GRP):
                    c0 = j * MICRO
                    nc.sync.dma_start_transpose(Bt[:, c0:c0 + MICRO],
                                                h1t[:, j, 0:128])
                gs["c1"] = c1t
                gs["h1"] = h1t

            def node_block(g):
                st = sstate[g]
                TTs, Bt, Dt = st["TT"], st["B"], st["D"]
                EK, CQt = st["EK"], st["CQt"]
                c1t = gstate[g]["c1"]
                h1t = gstate[g]["h1"]
                OUTt = opool.tile([128, GRP, 336], BF, tag="OUT")
                sgn = gpool.tile([128, GRP, 672], BF, tag="sgn")
                tu2 = gpool.tile([128, GRP, 168], BF, tag="tu2")
                for j in range(GRP):
                    c0 = j * MICRO
                    chunks = [(TTs[:, c0:c0 + MICRO], wttn),
                              (Bt[:, c0:c0 + MICRO], wb),
                              (EK[:, 4, c0:c0 + MICRO], wc),
                              (Dt[:, c0:c0 + MICRO], wd)]
                    P = pnpool.tile([128, 1024], F32, tag="psn")
                    # 40-row h1 tail transpose into bank-1 spare space,
                    # evacuated into the D chunk before the matmuls start.
                    pt = P[0:40, 896:960].bitcast(BF)
                    nc.tensor.transpose(pt, h1t[:, j, 128:168], ident[:])
                    nc.vector.tensor_copy(Dt[0:40, c0:c0 + MICRO], pt)
                    for ci, (X, W) in enumerate(chunks):
                        first, last = ci == 0, ci == len(chunks) - 1
                        nc.tensor.matmul(P[:, 0:504], X,
                                         W[:, 0:504], start=first, stop=last)
                        nc.tensor.matmul(P[:, 512:848], X,
                                         W[:, 504:840], start=first, stop=last)
                    pr = P[:].rearrange("p (a b) -> p a b", a=2, b=512)
                    sr = sgn[:, j, :].rearrange("p (a b) -> p a b", a=2, b=336)
                    nc.scalar.activation(sr, pr[:, :, 0:336], SIGF)
                    nc.scalar.activation(tu2[:, j, :], P[:, 336:504], TANHF)
                t3 = gpool.tile([128, GRP, 168], BF, tag="tmpA")
                nc.vector.tensor_mul(t3[:], sgn[:, :, 0:168], tu2[:])
                t4 = gpool.tile([128, GRP, 168], BF, tag="tmpB")
                nc.vector.tensor_mul(t4[:], sgn[:, :, 504:672],
                                     CQt[:, :, 168:336])
                t5 = gpool.tile([128, GRP, 168], BF, tag="tmpC")
                nc.vector.tensor_mul(t5[:], sgn[:, :, 336:504], c1t[:])
                t6 = gpool.tile([128, GRP, 168], BF, tag="tmpA")
                nc.vector.tensor_add(t6[:], t3[:], t4[:])
                nc.vector.tensor_add(OUTt[:, :, 168:336], t6[:], t5[:])
                tc2 = gpool.tile([128, GRP, 168], BF, tag="tc2")
                nc.scalar.activation(tc2[:], OUTt[:, :, 168:336], TANHF)
                nc.vector.tensor_mul(OUTt[:, :, 0:168],
                                     sgn[:, :, 168:336], tc2[:])
                return OUTt

            def store_group(g, OUTt):
                cs = g * GNODES
                nc.gpsimd.dma_start(
                    out_d[cs:cs + GNODES, :].rearrange("(m p) f -> p m f",
                                                       p=128),
                    OUTt[:])

            load_group(0)
            if ngrp > 1:
                load_group(1)
            for g in range(ngrp):
                leaf_block(g)
                if g >= 2:
                    store_group(g - 2, node_block(g - 2))
                chain_block(g)
                if g + 2 < ngrp:
                    load_group(g + 2)
            for g in (ngrp - 2, ngrp - 1):
                if g >= 0:
                    store_group(g, node_block(g))

    nc.compile()
    return nc


def _prep_core(inputs, c, npad=NPAD, nper=NPER):
    """Build the per-core (sharded, transposed, bf16) input arrays."""
    sl = slice(c * nper, (c + 1) * nper)
    e = inputs["e"][sl]
    h_prev = inputs["h_prev"][sl]
    tag = inputs["tag"][sl]
    tagp = inputs["tag_parent"][sl]
    k = inputs["k"][sl]
    c_prev = inputs["c_prev"][sl]
    q = inputs["q"][sl]
    n = e.shape[0]

    xbig = np.zeros((640, npad), BF16)
    xbig[0:300, :n] = e.T
    xbig[300:468, :n] = h_prev.T
    xbig[512:640, :n] = k[:, 0:128].T
    tt1T = np.zeros((101, npad), BF16)
    tt1T[0:50, :n] = tag.T
    tt1T[50:100, :n] = tagp.T
    tt1T[100, :n] = 1.0
    ktail = np.zeros((40, npad), BF16)
    ktail[:, :n] = k[:, 128:168].T
    cq = np.zeros((npad, 336), BF16)
    cq[:n, 0:168] = c_prev
    cq[:n, 168:336] = q
    return dict(xbig=xbig, tt1T=tt1T, ktail=ktail, cq=cq)


def _prep_weights(inputs):
    cat = np.concatenate
    w_eh = cat([inputs["We_l"], inputs["Uh_l"]], 0).astype(BF16)
    w_ttl = cat([inputs["Wt_l"], inputs["Wtp_l"], inputs["b_l"][None, :]],
                0).astype(BF16)
    # node gate order: [i2, o2, u2 | fl2, fd2] (source order i,o,fl,fd,u)
    perm = np.concatenate([np.arange(0, 336), np.arange(672, 840),
                           np.arange(336, 672)])
    w_ttn = cat([inputs["Wt_n"], inputs["Wtp_n"], inputs["b_n"][None, :]],
                0)[:, perm].astype(BF16)
    uh = inputs["Uh_n"][:, perm].astype(BF16)
    uk = np.zeros((168, 840), BF16)
    uk[:, 0:336] = inputs["Uk_n"][:, 0:336]      # i2, o2
    uk[:, 504:672] = inputs["Uk_n"][:, 336:504]  # fl2
    uk[:, 672:840] = inputs["Uk_n"][:, 504:672]  # fd2
    w_b = np.ascontiguousarray(uh[0:128])
    w_c = np.ascontiguousarray(uk[0:128])
    w_d = cat([uh[128:168], uk[128:168]], 0)
    return dict(w_eh=w_eh, w_ttl=w_ttl, w_ttn=w_ttn, w_b=w_b, w_c=w_c,
                w_d=w_d)


def kernel(**inputs):
    global _compiled, LAST_RESULT
    if _compiled is None:
        _compiled = _build()
    weights = _prep_weights(inputs)
    in_maps = []
    for c in range(N_CORES):
        m = _prep_core(inputs, c)
        m.update(weights)
        in_maps.append(m)
    res = run_bass_kernel_spmd(_compiled, in_maps,
                               core_ids=list(range(N_CORES)))
    LAST_RESULT = res
    outs = [res.results[c]["out"][:NPER].astype(np.float32)
            for c in range(N_CORES)]
    return np.concatenate(outs, 0)


# revision 15
# speedup vs baseline: 1.2162x; 1.2162x over previous
"""Trainium2 Bass kernel for the batched ConstituencyTreeLSTM cell.

Data-parallel across 8 NeuronCores: each core processes 12500 nodes
(padded to 12544 = 98 micro-tiles of 128 nodes, grouped 7 per "group").
Per 128-node micro-tile:

  leaf:  stationary chunks [e;h_prev]^T (4) + [tag;tagp;1]^T (1) vs fused
         weights [We;Uh] / [Wt;Wtp;b]  ->  bf16 PSUM [128, i|o|fl|u]
         c1 = sig(i)*tanh(u) + sig(fl)*c_prev ; h1 = sig(o)*tanh(c1)
  node:  h1^T obtained on-chip (rows 0:128 via DMA xbar transpose, rows
         128:168 via PE transpose); stationary chunks [tag;tagp;1]^T,
         h1^T, [h1^T;k^T], k^T (4) vs [Wt_n;Wtp_n;b_n] / Uh_n / Uk_n
         (u2 gate zero-padded in Uk)  ->  bf16 PSUM [128, i2|o2|fl2|fd2|u2]
         c2 = sig(i2)*tanh(u2) + sig(fd2)*q + sig(fl2)*c1
         h2 = sig(o2)*tanh(c2);  out = [h2|c2]

All matmul inputs are bf16 (bf16 PSUM accumulation); biases are folded in
via a ones-row. Elementwise runs bf16 on DVE, activations on ACT.
Pipeline: node block of group g runs two iterations after its leaf block
so the h1 -> transpose chain never stalls the TensorEngine (keeps HAM warm).
"""

import os
import sys

import numpy as np

try:
    import concourse.bass as bass  # noqa: F401
except Exception:  # pragma: no cover - fallback for bare environments
    for p in (
        "/root/.axon_site",
        "/root/.axon_site/_ro/trn_rl_repo",
        "/root/.axon_site/_ro/pypackages",
        "/opt/trn_rl_repo",
        "/opt/pypackages",
    ):
        if os.path.isdir(p) and p not in sys.path:
            sys.path.append(p)
    import concourse.bass as bass  # noqa: F401

import ml_dtypes
import concourse.mybir as mybir
import concourse.tile as tile
from concourse import bacc
from concourse.bass_utils import run_bass_kernel_spmd
from concourse.masks import make_identity

BF16 = ml_dtypes.bfloat16

N_CORES = 8
N = 100000
NPER = N // N_CORES            # 12500
MICRO = 128                    # nodes per matmul tile (stationary free dim)
GRP = 7                        # micro tiles per group
GNODES = MICRO * GRP           # 896
NGRP = 14                      # groups per core
NPAD = NGRP * GNODES           # 12544
M = 168                        # mem dim

ROWS_EH = [(0, 128), (128, 256), (256, 384), (384, 468)]  # [e;h_prev] chunks

F32 = mybir.dt.float32
BF = mybir.dt.bfloat16
SIGF = mybir.ActivationFunctionType.Sigmoid
TANHF = mybir.ActivationFunctionType.Tanh

_compiled = None
LAST_RESULT = None


def _build(ngrp=NGRP):
    npad = ngrp * GNODES
    nc = bacc.Bacc("TRN2", target_bir_lowering=False, debug=False,
                   num_devices=N_CORES)

    xbig_d = nc.dram_tensor("xbig", [640, npad], BF, kind="ExternalInput")
    tt1T_d = nc.dram_tensor("tt1T", [101, npad], BF, kind="ExternalInput")
    ktail_d = nc.dram_tensor("ktail", [40, npad], BF, kind="ExternalInput")
    cq_d = nc.dram_tensor("cq", [npad, 336], BF, kind="ExternalInput")
    w_eh_d = nc.dram_tensor("w_eh", [468, 672], BF, kind="ExternalInput")
    w_ttl_d = nc.dram_tensor("w_ttl", [101, 672], BF, kind="ExternalInput")
    w_ttn_d = nc.dram_tensor("w_ttn", [101, 840], BF, kind="ExternalInput")
    w_b_d = nc.dram_tensor("w_b", [128, 840], BF, kind="ExternalInput")
    w_c_d = nc.dram_tensor("w_c", [128, 840], BF, kind="ExternalInput")
    w_d_d = nc.dram_tensor("w_d", [80, 840], BF, kind="ExternalInput")
    out_d = nc.dram_tensor("out", [npad, 336], BF, kind="ExternalOutput")

    with tile.TileContext(nc) as tc:
        from contextlib import ExitStack
        with ExitStack() as ctx:
            wpool = ctx.enter_context(tc.tile_pool(name="w", bufs=1))
            spool = ctx.enter_context(tc.tile_pool(name="s", bufs=5))
            opool = ctx.enter_context(tc.tile_pool(name="o", bufs=2))
            gpool = ctx.enter_context(tc.tile_pool(name="g", bufs=2))
            c1pool = ctx.enter_context(tc.tile_pool(name="c1p", bufs=3))
            plpool = ctx.enter_context(
                tc.tile_pool(name="pl", bufs=2, space="PSUM"))
            pnpool = ctx.enter_context(
                tc.tile_pool(name="pn", bufs=2, space="PSUM"))

            # ---- constants / weights (resident) ----
            ident = wpool.tile([128, 128], BF, tag="ident")
            make_identity(nc, ident[:])
            wleaf = []
            for i, (r0, r1) in enumerate(ROWS_EH):
                t = wpool.tile([r1 - r0, 672], BF, tag=f"weh{i}")
                nc.gpsimd.dma_start(t[:], w_eh_d[r0:r1, :])
                wleaf.append(t)
            wttl = wpool.tile([101, 672], BF, tag="wttl")
            nc.gpsimd.dma_start(wttl[:], w_ttl_d[:, :])
            wttn = wpool.tile([101, 840], BF, tag="wttn")
            nc.gpsimd.dma_start(wttn[:], w_ttn_d[:, :])
            wb = wpool.tile([128, 840], BF, tag="wb")
            nc.gpsimd.dma_start(wb[:], w_b_d[:, :])
            wc = wpool.tile([128, 840], BF, tag="wc")
            nc.gpsimd.dma_start(wc[:], w_c_d[:, :])
            wd = wpool.tile([80, 840], BF, tag="wd")
            nc.gpsimd.dma_start(wd[:], w_d_d[:, :])

            sstate = {}
            gstate = {}

            def load_group(g):
                cs = g * GNODES
                EK = spool.tile([128, 5, GNODES], BF, tag="EK")
                nc.sync.dma_start(
                    EK[:],
                    xbig_d[:, cs:cs + GNODES].rearrange("(c p) n -> p c n",
                                                        p=128))
                TTs = spool.tile([101, GNODES], BF, tag="TT")
                nc.sync.dma_start(TTs[:], tt1T_d[:, cs:cs + GNODES])
                Dt = spool.tile([80, GNODES], BF, tag="D")
                nc.gpsimd.dma_start(Dt[40:80, :], ktail_d[:, cs:cs + GNODES])
                Bt = spool.tile([128, GNODES], BF, tag="B")
                CQt = spool.tile([128, GRP, 336], BF, tag="CQ")
                nc.gpsimd.dma_start(
                    CQt[:],
                    cq_d[cs:cs + GNODES, :].rearrange("(m p) f -> p m f",
                                                      p=128))
                sstate[g] = dict(EK=EK, TT=TTs, D=Dt, B=Bt, CQt=CQt)

            def leaf_block(g):
                st = sstate[g]
                EK, TTs = st["EK"], st["TT"]
                sgl = gpool.tile([128, GRP, 504], BF, tag="sgl")
                tul = gpool.tile([128, GRP, 168], BF, tag="tul")
                for j in range(GRP):
                    c0 = j * MICRO
                    chunks = [(EK[0:r1 - r0, i, c0:c0 + MICRO], wleaf[i])
                              for i, (r0, r1) in enumerate(ROWS_EH)]
                    chunks.append((TTs[:, c0:c0 + MICRO], wttl))
                    P = plpool.tile([128, 1024], F32, tag="psl")
                    for ci, (X, W) in enumerate(chunks):
                        first, last = ci == 0, ci == len(chunks) - 1
                        nc.tensor.matmul(P[:, 0:504], X,
                                         W[:, 0:504], start=first, stop=last)
                        nc.tensor.matmul(P[:, 512:680], X,
                                         W[:, 504:672], start=first, stop=last)
                    nc.scalar.activation(sgl[:, j, :], P[:, 0:504], SIGF)
                    nc.scalar.activation(tul[:, j, :], P[:, 512:680], TANHF)
                gstate[g] = dict(sgl=sgl, tul=tul)

            def chain_block(g):
                st = sstate[g]
                gs = gstate[g]
                sgl, tul = gs["sgl"], gs["tul"]
                CQt, Bt = st["CQt"], st["B"]
                t1 = gpool.tile([128, GRP, 168], BF, tag="tmpA")
                nc.vector.tensor_mul(t1[:], sgl[:, :, 0:168], tul[:])
                t2 = gpool.tile([128, GRP, 168], BF, tag="tmpB")
                nc.vector.tensor_mul(t2[:], sgl[:, :, 336:504],
                                     CQt[:, :, 0:168])
                c1t = c1pool.tile([128, GRP, 168], BF, tag="c1")
                nc.vector.tensor_add(c1t[:], t1[:], t2[:])
                tc1 = gpool.tile([128, GRP, 168], BF, tag="tc1")
                nc.scalar.activation(tc1[:], c1t[:], TANHF)
                h1t = c1pool.tile([128, GRP, 168], BF, tag="h1")
                nc.vector.tensor_mul(h1t[:], sgl[:, :, 168:336], tc1[:])
                if g >= 2:
                    for j in range(GRP):
                        c0 = j * MICRO
                        nc.sync.dma_start_transpose(Bt[:, c0:c0 + MICRO],
                                                    h1t[:, j, 0:128])
                gs["c1"] = c1t
                gs["h1"] = h1t

            def node_block(g):
                st = sstate[g]
                TTs, Bt, Dt = st["TT"], st["B"], st["D"]
                EK, CQt = st["EK"], st["CQt"]
                c1t = gstate[g]["c1"]
                h1t = gstate[g]["h1"]
                OUTt = opool.tile([128, GRP, 336], BF, tag="OUT")
                sgn = gpool.tile([128, GRP, 672], BF, tag="sgn")
                tu2 = gpool.tile([128, GRP, 168], BF, tag="tu2")
                for j in range(GRP):
                    c0 = j * MICRO
                    chunks = [(TTs[:, c0:c0 + MICRO], wttn),
                              (Bt[:, c0:c0 + MICRO], wb),
                              (EK[:, 4, c0:c0 + MICRO], wc),
                              (Dt[:, c0:c0 + MICRO], wd)]
                    P = pnpool.tile([128, 1024], F32, tag="psn")
                    # 40-row h1 tail transpose into bank-1 spare space,
                    # evacuated into the D chunk before the matmuls start.
                    pt = P[0:40, 896:960].bitcast(BF)
                    nc.tensor.transpose(pt, h1t[:, j, 128:168], ident[:])
                    nc.vector.tensor_copy(Dt[0:40, c0:c0 + MICRO], pt)
                    if g < 2:
                        # startup groups: B half transposed on PE too, so the
                        # sync ring's first xbar comes after the load burst
                        pb = P[0:128, 960:1024].bitcast(BF)
                        nc.tensor.transpose(pb, h1t[:, j, 0:128], ident[:])
                        nc.vector.tensor_copy(Bt[:, c0:c0 + MICRO], pb)
                    for ci, (X, W) in enumerate(chunks):
                        first, last = ci == 0, ci == len(chunks) - 1
                        nc.tensor.matmul(P[:, 0:504], X,
                                         W[:, 0:504], start=first, stop=last)
                        nc.tensor.matmul(P[:, 512:848], X,
                                         W[:, 504:840], start=first, stop=last)
                    pr = P[:].rearrange("p (a b) -> p a b", a=2, b=512)
                    sr = sgn[:, j, :].rearrange("p (a b) -> p a b", a=2, b=336)
                    nc.scalar.activation(sr, pr[:, :, 0:336], SIGF)
                    nc.scalar.activation(tu2[:, j, :], P[:, 336:504], TANHF)
                t3 = gpool.tile([128, GRP, 168], BF, tag="tmpA")
                nc.vector.tensor_mul(t3[:], sgn[:, :, 0:168], tu2[:])
                t4 = gpool.tile([128, GRP, 168], BF, tag="tmpB")
                nc.vector.tensor_mul(t4[:], sgn[:, :, 504:672],
                                     CQt[:, :, 168:336])
                t5 = gpool.tile([128, GRP, 168], BF, tag="tmpC")
                nc.vector.tensor_mul(t5[:], sgn[:, :, 336:504], c1t[:])
                t6 = gpool.tile([128, GRP, 168], BF, tag="tmpA")
                nc.vector.tensor_add(t6[:], t3[:], t4[:])
                nc.vector.tensor_add(OUTt[:, :, 168:336], t6[:], t5[:])
                tc2 = gpool.tile([128, GRP, 168], BF, tag="tc2")
                nc.scalar.activation(tc2[:], OUTt[:, :, 168:336], TANHF)
                nc.vector.tensor_mul(OUTt[:, :, 0:168],
                                     sgn[:, :, 168:336], tc2[:])
                return OUTt

            def store_group(g, OUTt):
                cs = g * GNODES
                nc.gpsimd.dma_start(
                    out_d[cs:cs + GNODES, :].rearrange("(m p) f -> p m f",
                                                       p=128),
                    OUTt[:])

            load_group(0)
            if ngrp > 1:
                load_group(1)
            for g in range(ngrp):
                leaf_block(g)
                if g >= 2:
                    store_group(g - 2, node_block(g - 2))
                chain_block(g)
                if g + 2 < ngrp:
                    load_group(g + 2)
            for g in (ngrp - 2, ngrp - 1):
                if g >= 0:
                    store_group(g, node_block(g))

    nc.compile()
    return nc


def _prep_core(inputs, c, npad=NPAD, nper=NPER):
    """Build the per-core (sharded, transposed, bf16) input arrays."""
    sl = slice(c * nper, (c + 1) * nper)
    e = inputs["e"][sl]
    h_prev = inputs["h_prev"][sl]
    tag = inputs["tag"][sl]
    tagp = inputs["tag_parent"][sl]
    k = inputs["k"][sl]
    c_prev = inputs["c_prev"][sl]
    q = inputs["q"][sl]
    n = e.shape[0]

    xbig = np.zeros((640, npad), BF16)
    xbig[0:300, :n] = e.T
    xbig[300:468, :n] = h_prev.T
    xbig[512:640, :n] = k[:, 0:128].T
    tt1T = np.zeros((101, npad), BF16)
    tt1T[0:50, :n] = tag.T
    tt1T[50:100, :n] = tagp.T
    tt1T[100, :n] = 1.0
    ktail = np.zeros((40, npad), BF16)
    ktail[:, :n] = k[:, 128:168].T
    cq = np.zeros((npad, 336), BF16)
    cq[:n, 0:168] = c_prev
    cq[:n, 168:336] = q
    return dict(xbig=xbig, tt1T=tt1T, ktail=ktail, cq=cq)


def _prep_weights(inputs):
    cat = np.concatenate
    w_eh = cat([inputs["We_l"], inputs["Uh_l"]], 0).astype(BF16)
    w_ttl = cat([inputs["Wt_l"], inputs["Wtp_l"], inputs["b_l"][None, :]],
                0).astype(BF16)
    # node gate order: [i2, o2, u2 | fl2, fd2] (source order i,o,fl,fd,u)
    perm = np.concatenate([np.arange(0, 336), np.arange(672, 840),
                           np.arange(336, 672)])
    w_ttn = cat([inputs["Wt_n"], inputs["Wtp_n"], inputs["b_n"][None, :]],
                0)[:, perm].astype(BF16)
    uh = inputs["Uh_n"][:, perm].astype(BF16)
    uk = np.zeros((168, 840), BF16)
    uk[:, 0:336] = inputs["Uk_n"][:, 0:336]      # i2, o2
    uk[:, 504:672] = inputs["Uk_n"][:, 336:504]  # fl2
    uk[:, 672:840] = inputs["Uk_n"][:, 504:672]  # fd2
    w_b = np.ascontiguousarray(uh[0:128])
    w_c = np.ascontiguousarray(uk[0:128])
    w_d = cat([uh[128:168], uk[128:168]], 0)
    return dict(w_eh=w_eh, w_ttl=w_ttl, w_ttn=w_ttn, w_b=w_b, w_c=w_c,
                w_d=w_d)


def kernel(**inputs):
    global _compiled, LAST_RESULT
    if _compiled is None:
        _compiled = _build()
    weights = _prep_weights(inputs)
    in_maps = []
    for c in range(N_CORES):
        m = _prep_core(inputs, c)
        m.update(weights)
        in_maps.append(m)
    res = run_bass_kernel_spmd(_compiled, in_maps,
                               core_ids=list(range(N_CORES)))
    LAST_RESULT = res
    outs = [res.results[c]["out"][:NPER].astype(np.float32)
            for c in range(N_CORES)]
    return np.concatenate(outs, 0)
